# revision 5
# baseline (speedup 1.0000x reference)
"""MoE grouped linear (DMoELinear) on 8 Trainium2 NeuronCores.

Expert-parallel sharding: tokens are sorted by expert id, so expert e's
tokens form one contiguous slice. Core e receives expert e's tokens
(padded to a uniform capacity C = max group size, so all cores run one
SPMD NEFF), expert e's weight (pre-transposed to [d_in, d_out]) and
bias, and computes
    yT_e = (x_e @ W_e.T).T.bf16 + b_e.bf16
with the weight block as the stationary matmul operand and tokens as
the moving free dim (so C needs no 128-alignment). The bias add fuses
into the PSUM-evacuation op as a per-partition scalar. The host does
all routing/gather in numpy.
"""

import numpy as np
import ml_dtypes

N_TOK, D_IN, D_OUT, N_EXP = 8192, 1024, 2048, 8
N_CORES = 8
P = 128
NFREE = 512  # max matmul moving free dim (one PSUM bank of f32)

BF16 = ml_dtypes.bfloat16

_nc_cache: dict[int, object] = {}


def _chunks(C):
    out = []
    off = 0
    while off < C:
        cw = min(NFREE, C - off)
        out.append((off, cw))
        off += cw
    return out


def _build_bass(C: int):
    """Emit the per-core Bass/Tile kernel for token capacity C."""
    import concourse.bass as bass  # noqa: F401  (registers engines)
    import concourse.mybir as mybir
    import concourse.tile as tile
    from concourse import bacc

    dt = mybir.dt
    KT = D_IN // P      # 8 contraction tiles
    DB = D_OUT // P     # 16 output-row blocks
    chunks = _chunks(C)

    nc = bacc.Bacc("TRN2", target_bir_lowering=False)

    xT_d = nc.dram_tensor("xT", [D_IN, C], dt.bfloat16, kind="ExternalInput")
    wT_d = nc.dram_tensor("wT", [D_IN, D_OUT], dt.bfloat16, kind="ExternalInput")
    bias_d = nc.dram_tensor("biasp", [P, DB], dt.float32, kind="ExternalInput")
    y_d = nc.dram_tensor("yT", [D_OUT, C], dt.bfloat16, kind="ExternalOutput")

    with tile.TileContext(nc) as tc:
        with (
            tc.tile_pool(name="persist", bufs=1) as ppool,
            tc.tile_pool(name="yout", bufs=3) as ypool,
            tc.tile_pool(name="psum", bufs=8, space="PSUM") as pspool,
        ):
            bt = ppool.tile([P, DB], dt.float32, name="bias", tag="bias")
            nc.sync.dma_start(bt[:], bias_d[:])
            x_tiles = []
            w_tiles = []
            for ki in range(KT):
                xt = ppool.tile([P, C], dt.bfloat16, name=f"x{ki}", tag=f"x{ki}")
                nc.sync.dma_start(xt[:], xT_d[ki * P:(ki + 1) * P, :])
                x_tiles.append(xt)
                wt = ppool.tile([P, D_OUT], dt.bfloat16, name=f"w{ki}", tag=f"w{ki}")
                nc.scalar.dma_start(wt[:], wT_d[ki * P:(ki + 1) * P, :])
                w_tiles.append(wt)

            ep = 0
            for db in range(DB):
                psums = [
                    pspool.tile([P, NFREE], dt.float32, name=f"ps{db}_{j}", tag="ps")
                    for j in range(len(chunks))
                ]
                for ki in range(KT):
                    lhsT = w_tiles[ki][:, db * P:(db + 1) * P]
                    for j, (off, cw) in enumerate(chunks):
                        nc.tensor.matmul(
                            psums[j][:, :cw],
                            lhsT,
                            x_tiles[ki][:, off:off + cw],
                            start=(ki == 0),
                            stop=(ki == KT - 1),
                        )
                ysb = ypool.tile([P, C], dt.bfloat16, name="ysb", tag="ysb")
                bias_col = bt[:, db:db + 1]
                for j, (off, cw) in enumerate(chunks):
                    # fused PSUM->bf16 cast + per-partition bias add,
                    # alternating ACT / DVE so neither engine bottlenecks
                    if ep % 2 == 0:
                        nc.scalar.add(ysb[:, off:off + cw], psums[j][:, :cw], bias_col)
                    else:
                        nc.vector.tensor_scalar_add(
                            ysb[:, off:off + cw], psums[j][:, :cw], bias_col
                        )
                    ep += 1
                nc.sync.dma_start(y_d[db * P:(db + 1) * P, :], ysb[:])

    nc.compile()
    return nc


def _run_spmd(in_maps, C, trace=False, trace_cores=None):
    from concourse.bass_utils import run_bass_kernel_spmd

    nc = _nc_cache.get(C)
    if nc is None:
        nc = _build_bass(C)
        _nc_cache[C] = nc
    return run_bass_kernel_spmd(
        nc,
        in_maps,
        core_ids=list(range(N_CORES)),
        trace=trace,
        trace_cores=trace_cores,
    )


def _prepare(x, weight, bias, ids_sorted):
    """Host-side routing: returns (in_maps, C, counts, starts)."""
    x = np.asarray(x)
    weight = np.asarray(weight)
    bias = np.asarray(bias)
    ids = np.asarray(ids_sorted)

    counts = np.bincount(ids, minlength=N_EXP).astype(np.int64)
    starts = np.zeros(N_EXP, dtype=np.int64)
    starts[1:] = np.cumsum(counts)[:-1]
    C = max(int(counts.max()), 1)

    xb = x.astype(BF16)
    in_maps = []
    for e in range(N_EXP):
        n_e = int(counts[e])
        xeT = np.zeros((D_IN, C), dtype=BF16)
        if n_e:
            xeT[:, :n_e] = xb[starts[e]:starts[e] + n_e].T
        weT = np.ascontiguousarray(weight[e].T).astype(BF16)  # [d_in, d_out]
        bp = np.ascontiguousarray(
            bias[e].astype(BF16).astype(np.float32).reshape(D_OUT // P, P).T
        )
        in_maps.append({"xT": xeT, "wT": weT, "biasp": bp})
    return in_maps, C, counts, starts


def _assemble(results, counts, starts):
    out = np.empty((N_TOK, D_OUT), dtype=BF16)
    for e in range(N_EXP):
        n_e = int(counts[e])
        if n_e:
            out[starts[e]:starts[e] + n_e] = results[e]["yT"][:, :n_e].T
    return out


def kernel(x, weight, bias, ids_sorted):
    in_maps, C, counts, starts = _prepare(x, weight, bias, ids_sorted)
    res = _run_spmd(in_maps, C)
    return _assemble(res.results, counts, starts)
